# revision 38
# baseline (speedup 1.0000x reference)
"""Trainium2 Bass kernel for nn_Adj_layer (pairwise-diff conv stack + BN +
softmax + top-k masking), data-parallel over the batch axis on 8 NeuronCores.

Self-contained: hardcodes all shapes. Needs the concourse toolchain on the
python path (stock location /opt/trn_rl_repo inside the TRN2 container).
"""

import os
import sys

for _p in ("/opt/trn_rl_repo", os.path.expanduser("~/.axon_site/_ro/trn_rl_repo")):
    if os.path.isdir(_p) and _p not in sys.path:
        sys.path.insert(0, _p)

import numpy as np

import concourse.bacc as bacc
import concourse.bass as bass
import concourse.mybir as mybir
import concourse.tile as tile

F32 = mybir.dt.float32
F32R = mybir.dt.float32r
F16 = mybir.dt.float16
BF16 = mybir.dt.bfloat16
AF = mybir.ActivationFunctionType
ALU = mybir.AluOpType

N_CORES = 8
B, V, D, H = 8, 201, 256, 128
NPIX = V * V                # 40401 pixels per batch element
NTOT = B * NPIX             # BN statistics population
K = 100                     # top-k
EPS = 1e-5
SLOPE = 0.01
CIN = [D, 2 * H, 2 * H, H]  # per-block input channels
COUT = [2 * H, 2 * H, H, H]

# T = |x_i - x_j| is symmetric in (i, j) and the conv stack is per-pixel, so
# only the upper triangle (j >= i) is computed; the logit matrix is mirrored
# before the row softmax. Row i (width V-i) pairs with row V-1-i (width i+1)
# for a constant 202-pixel chunk; two chunks form a 404-pixel group so the
# fp32r matmuls hit the >=256-wide full-rate path (1 cyc/row vs 4 for fp32).
NTRI = V * (V + 1) // 2     # 20301 upper-tri pixels (incl diag)
CHUNK = V + 1               # 202
GW = 2 * CHUNK              # 404: pixels per group
NGRP = 51                   # groups 0..49 are 404 px; group 50 is 101 px

# packed per-core input [128, NCOLS] (f32): x^T | conv weights | bn params |
# identity | strict-lower masks | upper(-incl-diag) masks
_XT0 = 0
_W0 = 2 * V                          # 402
_W1 = _W0 + 512                      # 914
_W2 = _W1 + 512                      # 1426
_W3 = _W2 + 256                      # 1682
_W4 = _W3 + 128                      # 1810
_P0 = _W4 + 1                        # 1811
_P1 = _P0 + 4                        # 1815
_P2 = _P1 + 4                        # 1819
_P3 = _P2 + 2                        # 1821
_IDC = _P3 + 2                       # 1823  128x128 identity (PE transpose)
_M0 = _IDC + 128                     # 1951  strict-lower mask rows 0..127
_M1 = _M0 + V                        # 2152  strict-lower mask rows 128..200
_UM0 = _M1 + V                       # 2353  upper-incl-diag mask rows 0..127
_UM1 = _UM0 + V                      # 2554  upper-incl-diag mask rows 128..200
NCOLS = _UM1 + V                     # 2755
_WOFF = [_W0, _W1, _W2, _W3]
_POFF = [_P0, _P1, _P2, _P3]


def _chunk_geom(i):
    """pixel offset, width, and row segments [(row, col0, width, pos)]"""
    if i < 100:
        wA = V - i
        return CHUNK * i, CHUNK, [(i, i, wA, 0), (200 - i, 200 - i, i + 1, wA)]
    return CHUNK * 100, 101, [(100, 100, 101, 0)]


def _group_geom(g):
    """pixel offset, width, and row segments for group g (chunks 2g, 2g+1)"""
    if g < 50:
        n0, _, segsA = _chunk_geom(2 * g)
        _, _, segsB = _chunk_geom(2 * g + 1)
        segs = segsA + [(r, c, w, p + CHUNK) for (r, c, w, p) in segsB]
        return n0, GW, segs
    return _chunk_geom(100)


def _build_nc(trace_scopes=False, repeat=1):
    """repeat>1 unrolls the whole body N times inside one NEFF — used only by
    the timing harness to expose true per-body device time above the ~1ms
    fixed per-dispatch overhead of the axon tunnel."""
    nc = bacc.Bacc("TRN2", target_bir_lowering=False, num_devices=N_CORES)

    # ---- external I/O (per-core) ----
    pk_d = nc.dram_tensor("pk", [128, NCOLS], F32, kind="ExternalInput")
    out_d = nc.dram_tensor("outb", [V, V], F32, kind="ExternalOutput")

    from contextlib import ExitStack
    with tile.TileContext(nc) as tc, ExitStack() as stack:
        dram = stack.enter_context(tc.tile_pool(name="dram", bufs=1, space="DRAM"))
        resid = stack.enter_context(tc.tile_pool(name="resid", bufs=1))
        psum = stack.enter_context(tc.tile_pool(name="psum", bufs=8, space="PSUM"))

        # internal DRAM: fp16 activation bounce buffers for blocks 0/1
        # (ct-planar: plane ct at column ct*NTRI + pixel), the slot-aligned
        # logit buffers, and the tiny AllReduce buffers. Logit slot layout:
        # u flat slot r in [0,101) holds U[r, c] at flat offset 202*r + c
        # (valid c >= r); v flat slot s in [0,100) holds U[101+s, c] at
        # offset 202*s + c (valid c >= 101+s). Garbage positions land in the
        # strict-lower region after reload and are masked there.
        y0d = dram.tile([128, 2 * NTRI], F16, tag="y0d", name="y0d")
        y1d = dram.tile([128, 2 * NTRI], F16, tag="y1d", name="y1d")
        u_d = dram.tile([1, 20480], F32, tag="ud", name="ud")
        v_d = dram.tile([1, 20480], F32, tag="vd", name="vd")
        ar_in = [dram.tile([128, 2 * (COUT[k] // 128)], F32, tag=f"arin{k}", name=f"arin{k}")
                 for k in range(4)]
        ar_out = [dram.tile([128, 2 * (COUT[k] // 128)], F32, tag=f"arout{k}", name=f"arout{k}")
                  for k in range(4)]

        # resident SBUF
        pksb = resid.tile([128, NCOLS], F32, tag="pk", name="pksb")
        # fp32r matmul operands must be produced pre-rounded to fp32r, so
        # the conv weights are copied once into an f32r-typed resident tile
        wsb = resid.tile([128, _P0 - _W0], F32R, tag="wsb", name="wsb")
        negx = resid.tile([128, 2 * V], F32, tag="negx", name="negx")
        y2sb = resid.tile([128, NTRI], F16, tag="y2sb", name="y2sb")
        y3sb = resid.tile([128, NTRI], F16, tag="y3sb", name="y3sb")
        sumc = [resid.tile([128, NGRP], F32, tag=f"sumc{ot}", name=f"sumc{ot}") for ot in range(2)]
        sumsqc = [resid.tile([128, NGRP], F32, tag=f"sumsqc{ot}", name=f"sumsqc{ot}") for ot in range(2)]
        s_sb = [resid.tile([128, COUT[k] // 128], F32, tag=f"s{k}", name=f"s_sb{k}") for k in range(4)]
        t_sb = [resid.tile([128, COUT[k] // 128], F32, tag=f"t{k}", name=f"t_sb{k}") for k in range(4)]
        ydiag = [resid.tile([128, COUT[k] // 128], F32, tag=f"yd{k}", name=f"ydiag{k}")
                 for k in range(4)]

        xT = pksb[:, _XT0:_XT0 + 2 * V]
        ident = pksb[:, _IDC:_IDC + 128]
        mask0 = pksb[:, _M0:_M0 + V]
        mask1 = pksb[:, _M1:_M1 + V]
        umask0 = pksb[:, _UM0:_UM0 + V]
        umask1 = pksb[:, _UM1:_UM1 + V]

        def finalize_stats(k, wsm):
            """Column-reduce group partials, mirror the triangle stats to the
            full V x V population (full = 2*tri - 201*ydiag), AllReduce across
            cores, then compute BN affine s/t for block k."""
            nt = COUT[k] // 128
            sred = work.tile([128, 2 * nt], F32, tag="sred")
            for ot in range(nt):
                nc.vector.tensor_reduce(sred[:, ot:ot + 1], sumc[ot][:, :NGRP],
                                        axis=mybir.AxisListType.X, op=ALU.add)
                nc.vector.tensor_reduce(sred[:, nt + ot:nt + ot + 1],
                                        sumsqc[ot][:, :NGRP],
                                        axis=mybir.AxisListType.X, op=ALU.add)
            sredc = work.tile([128, 2 * nt], F32, tag="sredc")
            if k == 0:
                # diag pixels of block 0 are exactly zero: full = 2*tri
                nc.vector.tensor_scalar_mul(sredc[:], sred[:], 2.0)
            else:
                corr = work.tile([128, 2 * nt], F32, tag="corr")
                nc.vector.tensor_scalar_mul(corr[:, 0:nt], ydiag[k][:], float(V))
                nc.vector.scalar_tensor_tensor(corr[:, nt:2 * nt], ydiag[k][:],
                                               float(V), ydiag[k][:],
                                               op0=ALU.mult, op1=ALU.mult)
                nc.vector.scalar_tensor_tensor(sredc[:], sred[:], 2.0, corr[:],
                                               op0=ALU.mult, op1=ALU.subtract)
            nc.gpsimd.dma_start(ar_in[k][:], sredc[:])
            if os.environ.get("ADJ_NO_COLLECTIVE"):
                nc.gpsimd.dma_start(ar_out[k][:], ar_in[k][:])
            else:
                nc.gpsimd.collective_compute(
                    "AllReduce", ALU.add, replica_groups=[list(range(N_CORES))],
                    ins=[ar_in[k][:].opt()],
                    outs=[ar_out[k][:].opt()])
            gst = work.tile([128, 2 * nt], F32, tag="gst")
            nc.gpsimd.dma_start(gst[:], ar_out[k][:])
            mean = work.tile([128, nt], F32, tag="bn_mean")
            ey2 = work.tile([128, nt], F32, tag="bn_ey2")
            var = work.tile([128, nt], F32, tag="bn_var")
            sd = work.tile([128, nt], F32, tag="bn_sd")
            rd = work.tile([128, nt], F32, tag="bn_rd")
            tmp = work.tile([128, nt], F32, tag="bn_tmp")
            inv_n = 1.0 / float(NTOT)
            nc.vector.tensor_scalar_mul(mean[:], gst[:, 0:nt], inv_n)
            nc.vector.tensor_scalar_mul(ey2[:], gst[:, nt:2 * nt], inv_n)
            nc.vector.tensor_tensor(var[:], mean[:], mean[:], op=ALU.mult)
            nc.vector.tensor_tensor(var[:], ey2[:], var[:], op=ALU.subtract)
            nc.vector.tensor_scalar_add(var[:], var[:], EPS)
            # rd = 1/sqrt(var) via bit-hack + 3 Newton steps, all on the DVE:
            # Sqrt/Ln live in different activation-table sets than
            # Identity/Lrelu/Exp, and each table swap costs ~1.3us on the
            # post-AllReduce critical path (4 finalizes x 2 swaps each)
            vi = work.tile([128, nt], mybir.dt.int32, tag="bn_vi")
            nc.vector.tensor_scalar(vi[:], var[:].bitcast(mybir.dt.int32),
                                    1, None, op0=ALU.arith_shift_right)
            nc.vector.tensor_scalar(rd[:].bitcast(mybir.dt.int32), vi[:],
                                    -1, 0x5f3759df, op0=ALU.mult, op1=ALU.add)
            for _ in range(3):
                nc.vector.tensor_tensor(sd[:], rd[:], rd[:], op=ALU.mult)
                nc.vector.tensor_tensor(sd[:], sd[:], var[:], op=ALU.mult)
                nc.vector.tensor_scalar(sd[:], sd[:], -0.5, 1.5,
                                        op0=ALU.mult, op1=ALU.add)
                nc.vector.tensor_tensor(rd[:], rd[:], sd[:], op=ALU.mult)
            g_ap = pksb[:, _POFF[k]:_POFF[k] + nt]
            be_ap = pksb[:, _POFF[k] + nt:_POFF[k] + 2 * nt]
            nc.vector.tensor_tensor(s_sb[k][:], g_ap, rd[:], op=ALU.mult)
            # t = be - mean * s   (conv bias cancels inside batch-norm)
            nc.vector.tensor_tensor(tmp[:], mean[:], s_sb[k][:], op=ALU.mult)
            nc.vector.tensor_tensor(t_sb[k][:], be_ap, tmp[:], op=ALU.subtract)

        for _rep in range(repeat):
          # 2-space rep level keeps the body diff minimal
          nc.sync.dma_start(pksb[:], pk_d[:])
          nc.vector.tensor_copy(wsb[:], pksb[:, _W0:_P0])
          nc.vector.tensor_scalar_mul(negx[:], xT, -1.0)
          with tc.tile_pool(name=f"work{_rep}", bufs=2) as work, \
               tc.tile_pool(name=f"wsm{_rep}", bufs=4) as wsm:
            zt = wsm.tile([128, 160], F32, tag="zt")
            nc.gpsimd.memset(zt[:], 0.0)
            nc.sync.dma_start(u_d[0:1, :], zt[:])
            nc.sync.dma_start(v_d[0:1, :], zt[:])

            def build_T(npx, segs):
                """T = |x_i - x_j|: per-seg differences on the DVE, then a
                whole-tile abs per ct half (DVE stt-max / Act Abs split; the
                one-pass abs_max/bitwise_and forms fail the walrus ISA
                check, and the abs pass is also where the mandatory f32r
                rounding of the matmul input happens)."""
                ttd = wsm.tile([128, 2 * GW], F32, tag="ttd")
                tt = wsm.tile([128, 2 * GW], F32R, tag="tt")
                for ct in range(2):
                    for (row, col0, w, pos) in segs:
                        nc.vector.tensor_scalar(
                            ttd[:, ct * npx + pos: ct * npx + pos + w],
                            xT[:, ct * V + col0: ct * V + col0 + w],
                            negx[:, ct * V + row: ct * V + row + 1],
                            None, op0=ALU.add)
                nc.vector.scalar_tensor_tensor(
                    tt[:, 0:npx], ttd[:, 0:npx], -1.0, ttd[:, 0:npx],
                    op0=ALU.mult, op1=ALU.max)
                nc.scalar.activation(tt[:, npx:2 * npx], ttd[:, npx:2 * npx],
                                     AF.Abs)
                return tt

            def conv(k, src_ap, npx, nti, nto):
                """fp32r matmuls: lhsT = f32r weights, rhs = f32r tiles.
                Odd (tail-group) widths are padded to even: fp32r matmuls
                with odd free size fail the walrus ISA check; the pad column
                of psum is never read."""
                mm = npx + (npx & 1)
                ps = [psum.tile([128, 512], F32, tag="ps", name="ps")
                      for _ in range(nto)]
                wct = COUT[k]
                w0 = _WOFF[k] - _W0
                for ot in range(nto):
                    for ct in range(nti):
                        nc.tensor.matmul(
                            ps[ot][:, :mm],
                            wsb[:, w0 + ct * wct + ot * 128:
                                 w0 + ct * wct + (ot + 1) * 128],
                            src_ap(ct, mm),
                            start=(ct == 0), stop=(ct == nti - 1))
                return ps

            def store_stats01(ps, g, npx, stage, on_act):
                """psum -> fp16 stage with fused per-channel sums (on the
                engines named by on_act, per-phase balance); squares on the
                Pool engine from the fp16 stage."""
                scr = wsm.tile([128, 2 * GW], F16, tag="scr01")
                for ot in range(2):
                    if on_act[ot]:
                        nc.scalar.activation(
                            stage[:, ot * npx: (ot + 1) * npx], ps[ot][:, :npx],
                            AF.Identity, accum_out=sumc[ot][:, g:g + 1])
                    else:
                        nc.vector.tensor_scalar(
                            stage[:, ot * npx: (ot + 1) * npx], ps[ot][:, :npx],
                            1.0, 0.0, op0=ALU.mult, op1=ALU.add,
                            accum_out=sumc[ot][:, g:g + 1])
                    if ot == 0 or not on_act[1]:
                        nc.vector.scalar_tensor_tensor(
                            scr[:, ot * npx:ot * npx + npx],
                            stage[:, ot * npx:ot * npx + npx], 1.0,
                            stage[:, ot * npx:ot * npx + npx],
                            op0=ALU.mult, op1=ALU.mult,
                            accum_out=sumsqc[ot][:, g:g + 1])
                    else:
                        nc.scalar.activation(
                            scr[:, npx:2 * npx], ps[1][:, :npx], AF.Square,
                            accum_out=sumsqc[1][:, g:g + 1])

            # ===== phase 0: T -> conv0 -> y0 (fp16, ct-planar) =============
            if trace_scopes:
                sc = nc.enter_named_scope("phase0")
            for g in range(NGRP):
                n0, npx, segs = _group_geom(g)
                tt = build_T(npx, segs)
                ps = conv(0, lambda ct, w: tt[:, ct * npx: ct * npx + w], npx, 2, 2)
                stage = wsm.tile([128, 2 * GW], F16, tag="st01")
                store_stats01(ps, g, npx, stage, on_act=(True, True))
                nc.gpsimd.dma_start(
                    y0d[:, :].rearrange("p (ct n) -> p ct n", ct=2)[:, :, n0:n0 + npx],
                    stage[:, 0:2 * npx].rearrange("p (ct n) -> p ct n", ct=2))
            finalize_stats(0, wsm)
            if trace_scopes:
                nc.leave_named_scope(sc)

            # ===== phase 1: z0 = lrelu(bn(y0)) -> conv1 -> y1 (fp16) =======
            # ===== phase 2: z1 -> conv2 -> y2 (fp16, SBUF-resident) ========
            for k in (1, 2):
                if trace_scopes:
                    sc = nc.enter_named_scope(f"phase{k}")
                nto = COUT[k] // 128
                ysrc_d = y0d if k == 1 else y1d
                yre = ysrc_d[:, :].rearrange("p (ct n) -> p ct n", ct=2)
                for p0g in range(0, NGRP, 4):
                    # group quad: one fused 2-plane load and one 1616-wide
                    # lrelu per ct amortize DMA-issue and the ~370ns
                    # per-instruction Act overhead
                    pair = list(range(p0g, min(p0g + 4, NGRP)))
                    p0 = _group_geom(p0g)[0]
                    w2 = sum(_group_geom(g)[1] for g in pair)
                    ysrc = work.tile([128, 8 * GW], F16, tag="yin")
                    nc.sync.dma_start(
                        ysrc[:, 0:2 * w2].rearrange("p (ct n) -> p ct n", ct=2),
                        yre[:, :, p0:p0 + w2])
                    z = work.tile([128, 8 * GW], F32R, tag="z")
                    for ct in range(2):
                        nc.scalar.activation(z[:, ct * w2: (ct + 1) * w2],
                                             ysrc[:, ct * w2: (ct + 1) * w2],
                                             AF.Lrelu,
                                             bias=t_sb[k - 1][:, ct:ct + 1],
                                             scale=s_sb[k - 1][:, ct:ct + 1],
                                             alpha=SLOPE)
                    for g in pair:
                        n0, npx, _ = _group_geom(g)
                        off = n0 - p0
                        ps = conv(k, lambda ct, w: z[:, ct * w2 + off:
                                                     ct * w2 + off + w],
                                  npx, 2, nto)
                        if k == 1:
                            stage = wsm.tile([128, 2 * GW], F16, tag="st01")
                            store_stats01(ps, g, npx, stage,
                                          on_act=(False, True))
                            for ot in range(2):
                                if g == 0:
                                    nc.vector.tensor_copy(
                                        ydiag[k][:, ot:ot + 1], ps[ot][:, 0:1])
                            nc.gpsimd.dma_start(
                                y1d[:, :].rearrange(
                                    "p (ct n) -> p ct n", ct=2)[:, :, n0:n0 + npx],
                                stage[:, 0:2 * npx].rearrange(
                                    "p (ct n) -> p ct n", ct=2))
                        else:
                            scr = wsm.tile([128, 2 * GW], F16, tag="scr01")
                            nc.vector.tensor_scalar(
                                y2sb[:, n0:n0 + npx], ps[0][:, :npx], 1.0,
                                0.0, op0=ALU.mult, op1=ALU.add,
                                accum_out=sumc[0][:, g:g + 1])
                            nc.vector.scalar_tensor_tensor(
                                scr[:, 0:npx], y2sb[:, n0:n0 + npx], 1.0,
                                y2sb[:, n0:n0 + npx],
                                op0=ALU.mult, op1=ALU.mult,
                                accum_out=sumsqc[0][:, g:g + 1])
                            if g == 0:
                                nc.vector.tensor_copy(ydiag[k][:, 0:1],
                                                      ps[0][:, 0:1])
                finalize_stats(k, wsm)
                if trace_scopes:
                    nc.leave_named_scope(sc)

            # ===== phase 3: z2 -> conv3 -> y3 (f32, SBUF-resident) =========
            if trace_scopes:
                sc = nc.enter_named_scope("phase3")
            for p0g in range(0, NGRP, 4):
                pair = list(range(p0g, min(p0g + 4, NGRP)))
                p0 = _group_geom(p0g)[0]
                w2 = sum(_group_geom(g)[1] for g in pair)
                z = work.tile([128, 4 * GW], F32R, tag="z34")
                nc.scalar.activation(z[:, :w2], y2sb[:, p0:p0 + w2],
                                     AF.Lrelu, bias=t_sb[2][:, 0:1],
                                     scale=s_sb[2][:, 0:1], alpha=SLOPE)
                for g in pair:
                    n0, npx, _ = _group_geom(g)
                    off = n0 - p0
                    ps = conv(3, lambda ct, w: z[:, off:off + w], npx, 1, 1)
                    scr = wsm.tile([128, 2 * GW], F16, tag="scr01")
                    if g % 2 == 0:
                        nc.scalar.activation(y3sb[:, n0:n0 + npx],
                                             ps[0][:, :npx], AF.Identity,
                                             accum_out=sumc[0][:, g:g + 1])
                    else:
                        nc.vector.tensor_scalar(
                            y3sb[:, n0:n0 + npx], ps[0][:, :npx], 1.0, 0.0,
                            op0=ALU.mult, op1=ALU.add,
                            accum_out=sumc[0][:, g:g + 1])
                    nc.vector.scalar_tensor_tensor(
                        scr[:, 0:npx], y3sb[:, n0:n0 + npx], 1.0,
                        y3sb[:, n0:n0 + npx], op0=ALU.mult, op1=ALU.mult,
                        accum_out=sumsqc[0][:, g:g + 1])
                    if g == 0:
                        nc.vector.tensor_copy(ydiag[3][:, 0:1], ps[0][:, 0:1])
            finalize_stats(3, wsm)
            if trace_scopes:
                nc.leave_named_scope(sc)

            # ===== phase 4: z3 -> logits, staged into slot-aligned u/v =====
            if trace_scopes:
                sc = nc.enter_named_scope("phase4")
            for p0g in range(0, NGRP, 4):
              pair = list(range(p0g, min(p0g + 4, NGRP)))
              p0 = _group_geom(p0g)[0]
              w2 = sum(_group_geom(g)[1] for g in pair)
              z = work.tile([128, 4 * GW], F32R, tag="z34")
              nc.scalar.activation(z[:, :w2], y3sb[:, p0:p0 + w2],
                                   AF.Lrelu, bias=t_sb[3][:, 0:1],
                                   scale=s_sb[3][:, 0:1], alpha=SLOPE)
              for g in pair:
                n0, npx, _ = _group_geom(g)
                off = n0 - p0
                lp = psum.tile([128, 512], F32, tag="ps", name="ps")
                mm = npx + (npx & 1)
                nc.tensor.matmul(lp[0:1, :mm],
                                 wsb[:, _W4 - _W0:_W4 - _W0 + 1],
                                 z[:, off:off + mm],
                                 start=True, stop=True)
                scp = wsm.tile([1, GW], F32, tag="scp")
                nc.vector.tensor_copy(scp[0:1, :npx], lp[0:1, :npx])
                if g < 50:
                    r0, r1 = 2 * g, 2 * g + 1          # seg-A rows
                    wA0, wA1 = V - r0, V - r1          # seg-A widths
                    # u slots r0, r1: U[r, c] at slot pos 202*r + c
                    nc.sync.dma_start(
                        u_d[0:1, CHUNK * r0 + r0:CHUNK * r0 + r0 + wA0],
                        scp[0:1, 0:wA0])
                    nc.sync.dma_start(
                        u_d[0:1, CHUNK * r1 + r1:CHUNK * r1 + r1 + wA1],
                        scp[0:1, CHUNK:CHUNK + wA1])
                    # v slot 98-2g holds row 199-2g (chunk 2g+1 seg B, width
                    # 2g+2 at pos c>=199-2g); slot 99-2g row 200-2g (2g+1)
                    nc.gpsimd.dma_start(
                        v_d[0:1, CHUNK * (98 - 2 * g) + 199 - 2 * g:
                            CHUNK * (98 - 2 * g) + V],
                        scp[0:1, CHUNK + wA1:2 * CHUNK])
                    nc.gpsimd.dma_start(
                        v_d[0:1, CHUNK * (99 - 2 * g) + 200 - 2 * g:
                            CHUNK * (99 - 2 * g) + V],
                        scp[0:1, wA0:CHUNK])
                else:
                    nc.sync.dma_start(
                        u_d[0:1, CHUNK * 100 + 100:CHUNK * 100 + 100 + 101],
                        scp[0:1, 0:101])
            if trace_scopes:
                nc.leave_named_scope(sc)

          # ===== phase 5: mirror logits, softmax + topk mask ===============
          if trace_scopes:
            sc = nc.enter_named_scope("phase5")
          NR = (K // 8) + 1  # 13 max8 rounds to reach rank 100
          with tc.tile_pool(name=f"smax{_rep}", bufs=2) as smax:
            nv = V - 128  # 73
            # reload U rows: partitions 0..100 from u slots, 101..200 from v
            ut = [smax.tile([128, V], F32, tag=f"ut{j}", name=f"ut{j}") for j in range(2)]
            u_re = u_d[0:1, 0:101 * CHUNK].rearrange("a (p c) -> (a p) c", c=CHUNK)
            v_re = v_d[0:1, 0:100 * CHUNK].rearrange("a (p c) -> (a p) c", c=CHUNK)
            nc.sync.dma_start(ut[0][0:101, :], u_re[0:101, 0:V])
            nc.sync.dma_start(ut[0][101:128, :], v_re[0:27, 0:V])
            nc.sync.dma_start(ut[1][0:nv, :], v_re[27:100, 0:V])
            # mask garbage (strict-lower positions) then mirror:
            # A = (U o umask) + strict_lower((U o umask)^T)
            utm = [smax.tile([128, V], F32, tag=f"utm{j}", name=f"utm{j}") for j in range(2)]
            nc.vector.tensor_tensor(utm[0][:, :], ut[0][:, :], umask0, op=ALU.mult)
            nc.vector.tensor_tensor(utm[1][:nv, :], ut[1][:nv, :],
                                    umask1[:nv, :], op=ALU.mult)
            at = [smax.tile([128, V], F32, tag=f"at{j}", name=f"at{j}") for j in range(2)]
            ptA = psum.tile([128, 512], F32, tag="ps", name="ps")
            ptB = psum.tile([128, 512], F32, tag="ps", name="ps")
            ptC = psum.tile([128, 512], F32, tag="ps", name="ps")
            ptD = psum.tile([128, 512], F32, tag="ps", name="ps")
            nc.tensor.transpose(ptA[:, 0:128], utm[0][:, 0:128], ident)
            nc.tensor.transpose(ptB[:, 0:nv], utm[1][0:nv, 0:128],
                                pksb[0:nv, _IDC:_IDC + nv])
            nc.tensor.transpose(ptC[0:nv, 0:128], utm[0][:, 128:V], ident)
            nc.tensor.transpose(ptD[0:nv, 0:nv], utm[1][0:nv, 128:V],
                                pksb[0:nv, _IDC:_IDC + nv])
            utt0 = smax.tile([128, V], F32, tag="utt0")
            utt1 = smax.tile([128, V], F32, tag="utt1")
            nc.vector.tensor_copy(utt0[:, 0:128], ptA[:, 0:128])
            nc.vector.tensor_copy(utt0[:, 128:V], ptB[:, 0:nv])
            nc.vector.tensor_copy(utt1[0:nv, 0:128], ptC[0:nv, 0:128])
            nc.vector.tensor_copy(utt1[0:nv, 128:V], ptD[0:nv, 0:nv])
            nc.gpsimd.tensor_tensor(utt0[:, :], utt0[:, :], mask0, op=ALU.mult)
            nc.vector.tensor_tensor(at[0][:, :], utm[0][:, :], utt0[:, :], op=ALU.add)
            nc.gpsimd.tensor_tensor(utt1[:nv, :], utt1[:nv, :], mask1[:nv, :],
                                    op=ALU.mult)
            nc.vector.tensor_tensor(at[1][:nv, :], utm[1][:nv, :],
                                    utt1[:nv, :], op=ALU.add)

            for rt, (r0, nr) in enumerate([(0, 128), (128, V - 128)]):
                lt = at[rt]
                lc = smax.tile([128, V], F32, tag="lc")
                nc.vector.tensor_copy(lc[:nr, :], lt[:nr, :])
                mx = smax.tile([128, 8 * NR], F32, tag="mx")
                for r in range(NR):
                    nc.vector.max(mx[:nr, 8 * r: 8 * (r + 1)], lc[:nr, :])
                    if r < NR - 1:
                        nc.vector.match_replace(lc[:nr, :],
                                                mx[:nr, 8 * r: 8 * (r + 1)],
                                                lc[:nr, :], -1e30)
                nmx = smax.tile([128, 1], F32, tag="nmx")
                nc.vector.tensor_scalar_mul(nmx[:nr, :], mx[:nr, 0:1], -1.0)
                et = smax.tile([128, V], F32, tag="et")
                rsum = smax.tile([128, 1], F32, tag="rsum")
                nc.scalar.activation(et[:nr, :], lt[:nr, :], AF.Exp,
                                     bias=nmx[:nr, 0:1], scale=1.0,
                                     accum_out=rsum[:nr, 0:1])
                rec = smax.tile([128, 1], F32, tag="rec")
                nc.vector.reciprocal(rec[:nr, :], rsum[:nr, :])
                pt = smax.tile([128, V], F32, tag="pt")
                nc.vector.tensor_scalar_mul(pt[:nr, :], et[:nr, :], rec[:nr, 0:1])
                ot_ = smax.tile([128, V], F32, tag="ot")
                nc.vector.scalar_tensor_tensor(ot_[:nr, :], lt[:nr, :],
                                               mx[:nr, K - 1:K], pt[:nr, :],
                                               op0=ALU.is_ge, op1=ALU.mult)
                nc.sync.dma_start(out_d[r0:r0 + nr, :], ot_[:nr, :])
          if trace_scopes:
            nc.leave_named_scope(sc)

    nc.finalize()
    return nc


def _prep_inputs(inputs):
    """Host-side packing of the full inputs into one [128, NCOLS] per-core
    array: x^T (core's batch element) | conv weights | BN params | masks."""
    x = np.ascontiguousarray(inputs["x"], dtype=np.float32)

    def ctile(w):  # [cout, cin] -> [128, cin/128 * cout] packed per cin-tile
        wT = np.ascontiguousarray(np.asarray(w, np.float32).T)  # [cin, cout]
        cin, cout = wT.shape
        return wT.reshape(cin // 128, 128, cout).transpose(1, 0, 2).reshape(128, -1)

    def pcols(v):  # [cout] -> [128, cout/128]
        return np.asarray(v, np.float32).reshape(-1, 128).T

    parts = [ctile(inputs[f"w{k}"]) for k in range(5)]
    for k in range(4):
        parts.append(np.concatenate(
            [pcols(inputs[f"g{k}"]), pcols(inputs[f"be{k}"])], axis=1))
    parts.append(np.eye(128, dtype=np.float32))
    p_idx = np.arange(128, dtype=np.float32)[:, None]
    c_idx = np.arange(V, dtype=np.float32)[None, :]
    parts.append((c_idx < p_idx).astype(np.float32))          # strict-lower rows 0..127
    parts.append((c_idx < p_idx + 128).astype(np.float32))    # strict-lower rows 128..200
    parts.append((c_idx >= p_idx).astype(np.float32))         # upper-incl-diag rows 0..127
    parts.append((c_idx >= p_idx + 128).astype(np.float32))   # upper-incl-diag rows 128..200
    shared = np.concatenate(parts, axis=1)
    assert shared.shape == (128, NCOLS - 2 * V), shared.shape

    in_maps = []
    for c in range(N_CORES):
        xt = x[c].T.reshape(2, 128, V).transpose(1, 0, 2).reshape(128, 2 * V)
        in_maps.append({"pk": np.ascontiguousarray(
            np.concatenate([xt, shared], axis=1), np.float32)})
    return in_maps


class _Runner:
    """Cached PJRT executor for the bass module.

    Functionally the same axon path as bass_utils.run_bass_kernel_spmd
    (shard_map over 8 neuron devices + bass_exec custom call), but the
    jitted executable is built ONCE and reused — run_bass_kernel_spmd
    rebuilds the jax.jit closure per call, paying a full retrace/relower
    (~700ms) on every invocation. The donated-zero-output trick is also
    dropped: this kernel writes every element of its output, so the
    dummy output-shaped operands can be persistent device arrays instead
    of per-call zero uploads.
    """

    def __init__(self, nc):
        import jax
        from jax.sharding import Mesh, PartitionSpec, NamedSharding
        from jax.experimental.shard_map import shard_map
        from concourse.bass2jax import (
            _bass_exec_p,
            partition_id_tensor,
            install_neuronx_cc_hook,
        )

        install_neuronx_cc_hook()
        self.jax = jax
        self.nc = nc
        if nc.dbg_addr is not None and nc.dbg_callbacks:
            raise RuntimeError("dbg callbacks unsupported under axon")
        self.dbg_name = nc.dbg_addr.name if nc.dbg_addr is not None else None

        partition_name = (
            nc.partition_id_tensor.name if nc.partition_id_tensor else None
        )
        in_names, out_names, out_avals, zero_shapes = [], [], [], []
        for alloc in nc.m.functions[0].allocations:
            if not isinstance(alloc, mybir.MemoryLocationSet):
                continue
            name = alloc.memorylocations[0].name
            if alloc.kind == "ExternalInput":
                if name != partition_name:
                    in_names.append(name)
            elif alloc.kind == "ExternalOutput":
                out_names.append(name)
                shape = tuple(alloc.tensor_shape)
                dtype = mybir.dt.np(alloc.dtype)
                out_avals.append(jax.core.ShapedArray(shape, dtype))
                zero_shapes.append((shape, dtype))
        n_params = len(in_names)
        n_outs = len(out_avals)
        all_in_names = list(in_names) + list(out_names)
        if partition_name is not None:
            all_in_names.append(partition_name)
        self.in_names, self.out_names, self.out_avals = in_names, out_names, out_avals

        devices = jax.devices()[:N_CORES]
        assert len(devices) == N_CORES
        self.mesh = Mesh(np.asarray(devices), ("core",))
        self.sharding = NamedSharding(self.mesh, PartitionSpec("core"))

        def _body(*args):
            operands = list(args)
            if partition_name is not None:
                operands.append(partition_id_tensor())
            return tuple(
                _bass_exec_p.bind(
                    *operands,
                    out_avals=tuple(out_avals),
                    in_names=tuple(all_in_names),
                    out_names=tuple(out_names),
                    lowering_input_output_aliases=(),
                    sim_require_finite=True,
                    sim_require_nnan=True,
                    nc=nc,
                )
            )

        self._sharded = jax.jit(
            shard_map(
                _body,
                mesh=self.mesh,
                in_specs=(PartitionSpec("core"),) * (n_params + n_outs),
                out_specs=(PartitionSpec("core"),) * n_outs,
                check_rep=False,
            ),
            keep_unused=True,
        )
        # persistent dummy operands for the output slots (never read: the
        # kernel fully writes its outputs)
        self._zeros = [
            jax.device_put(
                np.zeros((N_CORES * s[0],) + tuple(s[1:]), dt), self.sharding
            )
            for (s, dt) in zero_shapes
        ]

    def concat_inputs(self, in_maps):
        if self.dbg_name is not None:
            dbg = np.zeros((1, 2), np.uint32)
            in_maps = [{**m, self.dbg_name: dbg} for m in in_maps]
        return [
            np.concatenate([np.asarray(m[nm]) for m in in_maps], axis=0)
            for nm in self.in_names
        ]

    def put_inputs(self, concat_in):
        return [self.jax.device_put(a, self.sharding) for a in concat_in]

    def dispatch(self, dev_in):
        """one kernel execution on the 8 cores (async; returns device arrays)"""
        return self._sharded(*dev_in, *self._zeros)

    def run_full(self, in_maps):
        outs = self.dispatch(self.put_inputs(self.concat_inputs(in_maps)))
        n = N_CORES
        return [
            {
                nm: np.asarray(outs[i]).reshape(n, *self.out_avals[i].shape)[c]
                for i, nm in enumerate(self.out_names)
            }
            for c in range(n)
        ]


_RUNNER = None


def _get_runner():
    global _RUNNER
    if _RUNNER is None:
        _RUNNER = _Runner(_build_nc())
    return _RUNNER


def kernel(**inputs):
    r = _get_runner()
    res = r.run_full(_prep_inputs(inputs))
    return np.stack([res[c]["outb"] for c in range(N_CORES)], axis=0)


# revision 40
# speedup vs baseline: 1.5447x; 1.5447x over previous
"""Trainium2 Bass kernel for nn_Adj_layer (pairwise-diff conv stack + BN +
softmax + top-k masking), data-parallel over the batch axis on 8 NeuronCores.

Self-contained: hardcodes all shapes. Needs the concourse toolchain on the
python path (stock location /opt/trn_rl_repo inside the TRN2 container).
"""

import os
import sys

for _p in ("/opt/trn_rl_repo", os.path.expanduser("~/.axon_site/_ro/trn_rl_repo")):
    if os.path.isdir(_p) and _p not in sys.path:
        sys.path.insert(0, _p)

import numpy as np

import concourse.bacc as bacc
import concourse.bass as bass
import concourse.mybir as mybir
import concourse.tile as tile

F32 = mybir.dt.float32
F32R = mybir.dt.float32r
F16 = mybir.dt.float16
BF16 = mybir.dt.bfloat16
AF = mybir.ActivationFunctionType
ALU = mybir.AluOpType

N_CORES = 8
B, V, D, H = 8, 201, 256, 128
NPIX = V * V                # 40401 pixels per batch element
NTOT = B * NPIX             # BN statistics population
K = 100                     # top-k
EPS = 1e-5
SLOPE = 0.01
CIN = [D, 2 * H, 2 * H, H]  # per-block input channels
COUT = [2 * H, 2 * H, H, H]

# T = |x_i - x_j| is symmetric in (i, j) and the conv stack is per-pixel, so
# only the upper triangle (j >= i) is computed; the logit matrix is mirrored
# before the row softmax. Row i (width V-i) pairs with row V-1-i (width i+1)
# for a constant 202-pixel chunk; two chunks form a 404-pixel group so the
# fp32r matmuls hit the >=256-wide full-rate path (1 cyc/row vs 4 for fp32).
NTRI = V * (V + 1) // 2     # 20301 upper-tri pixels (incl diag)
CHUNK = V + 1               # 202
GW = 2 * CHUNK              # 404: pixels per group
NGRP = 51                   # groups 0..49 are 404 px; group 50 is 101 px

# packed per-core input [128, NCOLS] (f32): x^T | conv weights | bn params |
# identity | strict-lower masks | upper(-incl-diag) masks
_XT0 = 0
_W0 = 2 * V                          # 402
_W1 = _W0 + 512                      # 914
_W2 = _W1 + 512                      # 1426
_W3 = _W2 + 256                      # 1682
_W4 = _W3 + 128                      # 1810
_P0 = _W4 + 1                        # 1811
_P1 = _P0 + 4                        # 1815
_P2 = _P1 + 4                        # 1819
_P3 = _P2 + 2                        # 1821
_IDC = _P3 + 2                       # 1823  128x128 identity (PE transpose)
_M0 = _IDC + 128                     # 1951  strict-lower mask rows 0..127
_M1 = _M0 + V                        # 2152  strict-lower mask rows 128..200
_UM0 = _M1 + V                       # 2353  upper-incl-diag mask rows 0..127
_UM1 = _UM0 + V                      # 2554  upper-incl-diag mask rows 128..200
NCOLS = _UM1 + V                     # 2755
_WOFF = [_W0, _W1, _W2, _W3]
_POFF = [_P0, _P1, _P2, _P3]


def _chunk_geom(i):
    """pixel offset, width, and row segments [(row, col0, width, pos)]"""
    if i < 100:
        wA = V - i
        return CHUNK * i, CHUNK, [(i, i, wA, 0), (200 - i, 200 - i, i + 1, wA)]
    return CHUNK * 100, 101, [(100, 100, 101, 0)]


def _group_geom(g):
    """pixel offset, width, and row segments for group g (chunks 2g, 2g+1)"""
    if g < 50:
        n0, _, segsA = _chunk_geom(2 * g)
        _, _, segsB = _chunk_geom(2 * g + 1)
        segs = segsA + [(r, c, w, p + CHUNK) for (r, c, w, p) in segsB]
        return n0, GW, segs
    return _chunk_geom(100)


def _build_nc(trace_scopes=False, repeat=1):
    """repeat>1 unrolls the whole body N times inside one NEFF — used only by
    the timing harness to expose true per-body device time above the ~1ms
    fixed per-dispatch overhead of the axon tunnel."""
    nc = bacc.Bacc("TRN2", target_bir_lowering=False, num_devices=N_CORES)

    # ---- external I/O (per-core) ----
    pk_d = nc.dram_tensor("pk", [128, NCOLS], F32, kind="ExternalInput")
    out_d = nc.dram_tensor("outb", [V, V], F32, kind="ExternalOutput")

    from contextlib import ExitStack
    with tile.TileContext(nc) as tc, ExitStack() as stack:
        dram = stack.enter_context(tc.tile_pool(name="dram", bufs=1, space="DRAM"))
        resid = stack.enter_context(tc.tile_pool(name="resid", bufs=1))
        psum = stack.enter_context(tc.tile_pool(name="psum", bufs=8, space="PSUM"))

        # internal DRAM: fp16 activation bounce buffers for blocks 0/1
        # (ct-planar: plane ct at column ct*NTRI + pixel), the slot-aligned
        # logit buffers, and the tiny AllReduce buffers. Logit slot layout:
        # u flat slot r in [0,101) holds U[r, c] at flat offset 202*r + c
        # (valid c >= r); v flat slot s in [0,100) holds U[101+s, c] at
        # offset 202*s + c (valid c >= 101+s). Garbage positions land in the
        # strict-lower region after reload and are masked there.
        y0d = dram.tile([128, 2 * NTRI], F16, tag="y0d", name="y0d")
        y1d = dram.tile([128, 2 * NTRI], F16, tag="y1d", name="y1d")
        u_d = dram.tile([1, 20480], F32, tag="ud", name="ud")
        v_d = dram.tile([1, 20480], F32, tag="vd", name="vd")
        ar_in = [dram.tile([128, 2 * (COUT[k] // 128)], F32, tag=f"arin{k}", name=f"arin{k}")
                 for k in range(4)]
        ar_out = [dram.tile([128, 2 * (COUT[k] // 128)], F32, tag=f"arout{k}", name=f"arout{k}")
                  for k in range(4)]

        # resident SBUF
        pksb = resid.tile([128, NCOLS], F32, tag="pk", name="pksb")
        # fp32r matmul operands must be produced pre-rounded to fp32r, so
        # the conv weights are copied once into an f32r-typed resident tile
        wsb = resid.tile([128, _P0 - _W0], F32R, tag="wsb", name="wsb")
        negx = resid.tile([128, 2 * V], F32, tag="negx", name="negx")
        y2sb = resid.tile([128, NTRI], F16, tag="y2sb", name="y2sb")
        y3sb = resid.tile([128, NTRI], F16, tag="y3sb", name="y3sb")
        sumc = [resid.tile([128, NGRP], F32, tag=f"sumc{ot}", name=f"sumc{ot}") for ot in range(2)]
        sumsqc = [resid.tile([128, NGRP], F32, tag=f"sumsqc{ot}", name=f"sumsqc{ot}") for ot in range(2)]
        s_sb = [resid.tile([128, COUT[k] // 128], F32, tag=f"s{k}", name=f"s_sb{k}") for k in range(4)]
        t_sb = [resid.tile([128, COUT[k] // 128], F32, tag=f"t{k}", name=f"t_sb{k}") for k in range(4)]
        ydiag = [resid.tile([128, COUT[k] // 128], F32, tag=f"yd{k}", name=f"ydiag{k}")
                 for k in range(4)]

        xT = pksb[:, _XT0:_XT0 + 2 * V]
        ident = pksb[:, _IDC:_IDC + 128]
        mask0 = pksb[:, _M0:_M0 + V]
        mask1 = pksb[:, _M1:_M1 + V]
        umask0 = pksb[:, _UM0:_UM0 + V]
        umask1 = pksb[:, _UM1:_UM1 + V]

        def finalize_stats(k, wsm):
            """Column-reduce group partials, mirror the triangle stats to the
            full V x V population (full = 2*tri - 201*ydiag), AllReduce across
            cores, then compute BN affine s/t for block k."""
            nt = COUT[k] // 128
            sred = work.tile([128, 2 * nt], F32, tag="sred")
            for ot in range(nt):
                nc.vector.tensor_reduce(sred[:, ot:ot + 1], sumc[ot][:, :NGRP],
                                        axis=mybir.AxisListType.X, op=ALU.add)
                nc.vector.tensor_reduce(sred[:, nt + ot:nt + ot + 1],
                                        sumsqc[ot][:, :NGRP],
                                        axis=mybir.AxisListType.X, op=ALU.add)
            sredc = work.tile([128, 2 * nt], F32, tag="sredc")
            if k == 0:
                # diag pixels of block 0 are exactly zero: full = 2*tri
                nc.vector.tensor_scalar_mul(sredc[:], sred[:], 2.0)
            else:
                corr = work.tile([128, 2 * nt], F32, tag="corr")
                nc.vector.tensor_scalar_mul(corr[:, 0:nt], ydiag[k][:], float(V))
                nc.vector.scalar_tensor_tensor(corr[:, nt:2 * nt], ydiag[k][:],
                                               float(V), ydiag[k][:],
                                               op0=ALU.mult, op1=ALU.mult)
                nc.vector.scalar_tensor_tensor(sredc[:], sred[:], 2.0, corr[:],
                                               op0=ALU.mult, op1=ALU.subtract)
            nc.gpsimd.dma_start(ar_in[k][:], sredc[:])
            if os.environ.get("ADJ_NO_COLLECTIVE"):
                nc.gpsimd.dma_start(ar_out[k][:], ar_in[k][:])
            else:
                nc.gpsimd.collective_compute(
                    "AllReduce", ALU.add, replica_groups=[list(range(N_CORES))],
                    ins=[ar_in[k][:].opt()],
                    outs=[ar_out[k][:].opt()])
            gst = work.tile([128, 2 * nt], F32, tag="gst")
            nc.gpsimd.dma_start(gst[:], ar_out[k][:])
            mean = work.tile([128, nt], F32, tag="bn_mean")
            ey2 = work.tile([128, nt], F32, tag="bn_ey2")
            var = work.tile([128, nt], F32, tag="bn_var")
            sd = work.tile([128, nt], F32, tag="bn_sd")
            rd = work.tile([128, nt], F32, tag="bn_rd")
            tmp = work.tile([128, nt], F32, tag="bn_tmp")
            inv_n = 1.0 / float(NTOT)
            nc.vector.tensor_scalar_mul(mean[:], gst[:, 0:nt], inv_n)
            nc.vector.tensor_scalar_mul(ey2[:], gst[:, nt:2 * nt], inv_n)
            nc.vector.tensor_tensor(var[:], mean[:], mean[:], op=ALU.mult)
            nc.vector.tensor_tensor(var[:], ey2[:], var[:], op=ALU.subtract)
            nc.vector.tensor_scalar_add(var[:], var[:], EPS)
            # rd = 1/sqrt(var) via bit-hack + 3 Newton steps, all on the DVE:
            # Sqrt/Ln live in different activation-table sets than
            # Identity/Lrelu/Exp, and each table swap costs ~1.3us on the
            # post-AllReduce critical path (4 finalizes x 2 swaps each)
            vi = work.tile([128, nt], mybir.dt.int32, tag="bn_vi")
            nc.vector.tensor_scalar(vi[:], var[:].bitcast(mybir.dt.int32),
                                    1, None, op0=ALU.arith_shift_right)
            nc.vector.tensor_scalar(rd[:].bitcast(mybir.dt.int32), vi[:],
                                    -1, 0x5f3759df, op0=ALU.mult, op1=ALU.add)
            for _ in range(3):
                nc.vector.tensor_tensor(sd[:], rd[:], rd[:], op=ALU.mult)
                nc.vector.tensor_tensor(sd[:], sd[:], var[:], op=ALU.mult)
                nc.vector.tensor_scalar(sd[:], sd[:], -0.5, 1.5,
                                        op0=ALU.mult, op1=ALU.add)
                nc.vector.tensor_tensor(rd[:], rd[:], sd[:], op=ALU.mult)
            g_ap = pksb[:, _POFF[k]:_POFF[k] + nt]
            be_ap = pksb[:, _POFF[k] + nt:_POFF[k] + 2 * nt]
            nc.vector.tensor_tensor(s_sb[k][:], g_ap, rd[:], op=ALU.mult)
            # t = be - mean * s   (conv bias cancels inside batch-norm)
            nc.vector.tensor_tensor(tmp[:], mean[:], s_sb[k][:], op=ALU.mult)
            nc.vector.tensor_tensor(t_sb[k][:], be_ap, tmp[:], op=ALU.subtract)

        for _rep in range(repeat):
          # 2-space rep level keeps the body diff minimal
          nc.sync.dma_start(pksb[:], pk_d[:])
          nc.vector.tensor_copy(wsb[:], pksb[:, _W0:_P0])
          nc.vector.tensor_scalar_mul(negx[:], xT, -1.0)
          with tc.tile_pool(name=f"work{_rep}", bufs=2) as work, \
               tc.tile_pool(name=f"wsm{_rep}", bufs=4) as wsm:
            zt = wsm.tile([128, 160], F32, tag="zt")
            nc.gpsimd.memset(zt[:], 0.0)
            nc.sync.dma_start(u_d[0:1, :], zt[:])
            nc.sync.dma_start(v_d[0:1, :], zt[:])

            def build_T(npx, segs):
                """T = |x_i - x_j|: per-seg differences on the DVE, then a
                whole-tile abs per ct half (DVE stt-max / Act Abs split; the
                one-pass abs_max/bitwise_and forms fail the walrus ISA
                check, and the abs pass is also where the mandatory f32r
                rounding of the matmul input happens)."""
                ttd = wsm.tile([128, 2 * GW], F32, tag="ttd")
                tt = wsm.tile([128, 2 * GW], F32R, tag="tt")
                for ct in range(2):
                    for (row, col0, w, pos) in segs:
                        nc.vector.tensor_scalar(
                            ttd[:, ct * npx + pos: ct * npx + pos + w],
                            xT[:, ct * V + col0: ct * V + col0 + w],
                            negx[:, ct * V + row: ct * V + row + 1],
                            None, op0=ALU.add)
                nc.vector.scalar_tensor_tensor(
                    tt[:, 0:npx], ttd[:, 0:npx], -1.0, ttd[:, 0:npx],
                    op0=ALU.mult, op1=ALU.max)
                nc.scalar.activation(tt[:, npx:2 * npx], ttd[:, npx:2 * npx],
                                     AF.Abs)
                return tt

            def conv(k, src_ap, npx, nti, nto):
                """fp32r matmuls: lhsT = f32r weights, rhs = f32r tiles.
                Odd (tail-group) widths are padded to even: fp32r matmuls
                with odd free size fail the walrus ISA check; the pad column
                of psum is never read."""
                mm = npx + (npx & 1)
                ps = [psum.tile([128, 512], F32, tag="ps", name="ps")
                      for _ in range(nto)]
                wct = COUT[k]
                w0 = _WOFF[k] - _W0
                for ot in range(nto):
                    for ct in range(nti):
                        nc.tensor.matmul(
                            ps[ot][:, :mm],
                            wsb[:, w0 + ct * wct + ot * 128:
                                 w0 + ct * wct + (ot + 1) * 128],
                            src_ap(ct, mm),
                            start=(ct == 0), stop=(ct == nti - 1))
                return ps

            def store_stats01(ps, g, npx, stage, on_act):
                """psum -> fp16 stage with fused per-channel sums (on the
                engines named by on_act, per-phase balance); squares on the
                Pool engine from the fp16 stage."""
                scr = wsm.tile([128, 2 * GW], F16, tag="scr01")
                for ot in range(2):
                    if on_act[ot]:
                        nc.scalar.activation(
                            stage[:, ot * npx: (ot + 1) * npx], ps[ot][:, :npx],
                            AF.Identity, accum_out=sumc[ot][:, g:g + 1])
                    else:
                        nc.vector.tensor_scalar(
                            stage[:, ot * npx: (ot + 1) * npx], ps[ot][:, :npx],
                            1.0, 0.0, op0=ALU.mult, op1=ALU.add,
                            accum_out=sumc[ot][:, g:g + 1])
                    if ot == 0 or not on_act[1]:
                        nc.vector.scalar_tensor_tensor(
                            scr[:, ot * npx:ot * npx + npx],
                            stage[:, ot * npx:ot * npx + npx], 1.0,
                            stage[:, ot * npx:ot * npx + npx],
                            op0=ALU.mult, op1=ALU.mult,
                            accum_out=sumsqc[ot][:, g:g + 1])
                    else:
                        nc.scalar.activation(
                            scr[:, npx:2 * npx], ps[1][:, :npx], AF.Square,
                            accum_out=sumsqc[1][:, g:g + 1])

            # ===== phase 0: T -> conv0 -> y0 (fp16, ct-planar) =============
            if trace_scopes:
                sc = nc.enter_named_scope("phase0")
            for g in range(NGRP):
                n0, npx, segs = _group_geom(g)
                tt = build_T(npx, segs)
                ps = conv(0, lambda ct, w: tt[:, ct * npx: ct * npx + w], npx, 2, 2)
                stage = wsm.tile([128, 2 * GW], F16, tag="st01")
                store_stats01(ps, g, npx, stage, on_act=(True, True))
                nc.gpsimd.dma_start(
                    y0d[:, :].rearrange("p (ct n) -> p ct n", ct=2)[:, :, n0:n0 + npx],
                    stage[:, 0:2 * npx].rearrange("p (ct n) -> p ct n", ct=2))
            finalize_stats(0, wsm)
            if trace_scopes:
                nc.leave_named_scope(sc)

            # ===== phase 1: z0 = lrelu(bn(y0)) -> conv1 -> y1 (fp16) =======
            # ===== phase 2: z1 -> conv2 -> y2 (fp16, SBUF-resident) ========
            for k in (1, 2):
                if trace_scopes:
                    sc = nc.enter_named_scope(f"phase{k}")
                nto = COUT[k] // 128
                ysrc_d = y0d if k == 1 else y1d
                yre = ysrc_d[:, :].rearrange("p (ct n) -> p ct n", ct=2)
                for p0g in range(0, NGRP, 4):
                    # group quad: one fused 2-plane load and one 1616-wide
                    # lrelu per ct amortize DMA-issue and the ~370ns
                    # per-instruction Act overhead
                    pair = list(range(p0g, min(p0g + 4, NGRP)))
                    p0 = _group_geom(p0g)[0]
                    w2 = sum(_group_geom(g)[1] for g in pair)
                    ysrc = work.tile([128, 8 * GW], F16, tag="yin")
                    nc.sync.dma_start(
                        ysrc[:, 0:2 * w2].rearrange("p (ct n) -> p ct n", ct=2),
                        yre[:, :, p0:p0 + w2])
                    z = work.tile([128, 8 * GW], F32R, tag="z")
                    for ct in range(2):
                        nc.scalar.activation(z[:, ct * w2: (ct + 1) * w2],
                                             ysrc[:, ct * w2: (ct + 1) * w2],
                                             AF.Lrelu,
                                             bias=t_sb[k - 1][:, ct:ct + 1],
                                             scale=s_sb[k - 1][:, ct:ct + 1],
                                             alpha=SLOPE)
                    for g in pair:
                        n0, npx, _ = _group_geom(g)
                        off = n0 - p0
                        ps = conv(k, lambda ct, w: z[:, ct * w2 + off:
                                                     ct * w2 + off + w],
                                  npx, 2, nto)
                        if k == 1:
                            stage = wsm.tile([128, 2 * GW], F16, tag="st01")
                            store_stats01(ps, g, npx, stage,
                                          on_act=(False, True))
                            for ot in range(2):
                                if g == 0:
                                    nc.vector.tensor_copy(
                                        ydiag[k][:, ot:ot + 1], ps[ot][:, 0:1])
                            nc.gpsimd.dma_start(
                                y1d[:, :].rearrange(
                                    "p (ct n) -> p ct n", ct=2)[:, :, n0:n0 + npx],
                                stage[:, 0:2 * npx].rearrange(
                                    "p (ct n) -> p ct n", ct=2))
                        else:
                            scr = wsm.tile([128, 2 * GW], F16, tag="scr01")
                            nc.vector.tensor_scalar(
                                y2sb[:, n0:n0 + npx], ps[0][:, :npx], 1.0,
                                0.0, op0=ALU.mult, op1=ALU.add,
                                accum_out=sumc[0][:, g:g + 1])
                            nc.vector.scalar_tensor_tensor(
                                scr[:, 0:npx], y2sb[:, n0:n0 + npx], 1.0,
                                y2sb[:, n0:n0 + npx],
                                op0=ALU.mult, op1=ALU.mult,
                                accum_out=sumsqc[0][:, g:g + 1])
                            if g == 0:
                                nc.vector.tensor_copy(ydiag[k][:, 0:1],
                                                      ps[0][:, 0:1])
                finalize_stats(k, wsm)
                if trace_scopes:
                    nc.leave_named_scope(sc)

            # ===== phase 3: z2 -> conv3 -> y3 (f32, SBUF-resident) =========
            if trace_scopes:
                sc = nc.enter_named_scope("phase3")
            for p0g in range(0, NGRP, 4):
                pair = list(range(p0g, min(p0g + 4, NGRP)))
                p0 = _group_geom(p0g)[0]
                w2 = sum(_group_geom(g)[1] for g in pair)
                z = work.tile([128, 4 * GW], F32R, tag="z34")
                nc.scalar.activation(z[:, :w2], y2sb[:, p0:p0 + w2],
                                     AF.Lrelu, bias=t_sb[2][:, 0:1],
                                     scale=s_sb[2][:, 0:1], alpha=SLOPE)
                for g in pair:
                    n0, npx, _ = _group_geom(g)
                    off = n0 - p0
                    ps = conv(3, lambda ct, w: z[:, off:off + w], npx, 1, 1)
                    scr = wsm.tile([128, 2 * GW], F16, tag="scr01")
                    if g % 2 == 0:
                        nc.scalar.activation(y3sb[:, n0:n0 + npx],
                                             ps[0][:, :npx], AF.Identity,
                                             accum_out=sumc[0][:, g:g + 1])
                    else:
                        nc.vector.tensor_scalar(
                            y3sb[:, n0:n0 + npx], ps[0][:, :npx], 1.0, 0.0,
                            op0=ALU.mult, op1=ALU.add,
                            accum_out=sumc[0][:, g:g + 1])
                    nc.vector.scalar_tensor_tensor(
                        scr[:, 0:npx], y3sb[:, n0:n0 + npx], 1.0,
                        y3sb[:, n0:n0 + npx], op0=ALU.mult, op1=ALU.mult,
                        accum_out=sumsqc[0][:, g:g + 1])
                    if g == 0:
                        nc.vector.tensor_copy(ydiag[3][:, 0:1], ps[0][:, 0:1])
            finalize_stats(3, wsm)
            if trace_scopes:
                nc.leave_named_scope(sc)

            # ===== phase 4: z3 -> logits, staged into slot-aligned u/v =====
            if trace_scopes:
                sc = nc.enter_named_scope("phase4")
            for p0g in range(0, NGRP, 4):
              pair = list(range(p0g, min(p0g + 4, NGRP)))
              p0 = _group_geom(p0g)[0]
              w2 = sum(_group_geom(g)[1] for g in pair)
              z = work.tile([128, 4 * GW], F32R, tag="z34")
              nc.scalar.activation(z[:, :w2], y3sb[:, p0:p0 + w2],
                                   AF.Lrelu, bias=t_sb[3][:, 0:1],
                                   scale=s_sb[3][:, 0:1], alpha=SLOPE)
              for g in pair:
                n0, npx, _ = _group_geom(g)
                off = n0 - p0
                lp = psum.tile([128, 512], F32, tag="ps", name="ps")
                mm = npx + (npx & 1)
                nc.tensor.matmul(lp[0:1, :mm],
                                 wsb[:, _W4 - _W0:_W4 - _W0 + 1],
                                 z[:, off:off + mm],
                                 start=True, stop=True)
                scp = wsm.tile([1, GW], F32, tag="scp")
                nc.vector.tensor_copy(scp[0:1, :npx], lp[0:1, :npx])
                if g < 50:
                    r0, r1 = 2 * g, 2 * g + 1          # seg-A rows
                    wA0, wA1 = V - r0, V - r1          # seg-A widths
                    # u slots r0, r1: U[r, c] at slot pos 202*r + c
                    nc.sync.dma_start(
                        u_d[0:1, CHUNK * r0 + r0:CHUNK * r0 + r0 + wA0],
                        scp[0:1, 0:wA0])
                    nc.sync.dma_start(
                        u_d[0:1, CHUNK * r1 + r1:CHUNK * r1 + r1 + wA1],
                        scp[0:1, CHUNK:CHUNK + wA1])
                    # v slot 98-2g holds row 199-2g (chunk 2g+1 seg B, width
                    # 2g+2 at pos c>=199-2g); slot 99-2g row 200-2g (2g+1)
                    nc.gpsimd.dma_start(
                        v_d[0:1, CHUNK * (98 - 2 * g) + 199 - 2 * g:
                            CHUNK * (98 - 2 * g) + V],
                        scp[0:1, CHUNK + wA1:2 * CHUNK])
                    nc.gpsimd.dma_start(
                        v_d[0:1, CHUNK * (99 - 2 * g) + 200 - 2 * g:
                            CHUNK * (99 - 2 * g) + V],
                        scp[0:1, wA0:CHUNK])
                else:
                    nc.sync.dma_start(
                        u_d[0:1, CHUNK * 100 + 100:CHUNK * 100 + 100 + 101],
                        scp[0:1, 0:101])
            if trace_scopes:
                nc.leave_named_scope(sc)

          # ===== phase 5: mirror logits, softmax + topk mask ===============
          if trace_scopes:
            sc = nc.enter_named_scope("phase5")
          NR = (K // 8) + 1  # 13 max8 rounds to reach rank 100
          with tc.tile_pool(name=f"smax{_rep}", bufs=2) as smax:
            nv = V - 128  # 73
            # reload U rows: partitions 0..100 from u slots, 101..200 from v
            ut = [smax.tile([128, V], F32, tag=f"ut{j}", name=f"ut{j}") for j in range(2)]
            u_re = u_d[0:1, 0:101 * CHUNK].rearrange("a (p c) -> (a p) c", c=CHUNK)
            v_re = v_d[0:1, 0:100 * CHUNK].rearrange("a (p c) -> (a p) c", c=CHUNK)
            nc.sync.dma_start(ut[0][0:101, :], u_re[0:101, 0:V])
            nc.sync.dma_start(ut[0][101:128, :], v_re[0:27, 0:V])
            nc.sync.dma_start(ut[1][0:nv, :], v_re[27:100, 0:V])
            # mask garbage (strict-lower positions) then mirror:
            # A = (U o umask) + strict_lower((U o umask)^T)
            utm = [smax.tile([128, V], F32, tag=f"utm{j}", name=f"utm{j}") for j in range(2)]
            nc.vector.tensor_tensor(utm[0][:, :], ut[0][:, :], umask0, op=ALU.mult)
            nc.vector.tensor_tensor(utm[1][:nv, :], ut[1][:nv, :],
                                    umask1[:nv, :], op=ALU.mult)
            at = [smax.tile([128, V], F32, tag=f"at{j}", name=f"at{j}") for j in range(2)]
            ptA = psum.tile([128, 512], F32, tag="ps", name="ps")
            ptB = psum.tile([128, 512], F32, tag="ps", name="ps")
            ptC = psum.tile([128, 512], F32, tag="ps", name="ps")
            ptD = psum.tile([128, 512], F32, tag="ps", name="ps")
            nc.tensor.transpose(ptA[:, 0:128], utm[0][:, 0:128], ident)
            nc.tensor.transpose(ptB[:, 0:nv], utm[1][0:nv, 0:128],
                                pksb[0:nv, _IDC:_IDC + nv])
            nc.tensor.transpose(ptC[0:nv, 0:128], utm[0][:, 128:V], ident)
            nc.tensor.transpose(ptD[0:nv, 0:nv], utm[1][0:nv, 128:V],
                                pksb[0:nv, _IDC:_IDC + nv])
            utt0 = smax.tile([128, V], F32, tag="utt0")
            utt1 = smax.tile([128, V], F32, tag="utt1")
            nc.vector.tensor_copy(utt0[:, 0:128], ptA[:, 0:128])
            nc.vector.tensor_copy(utt0[:, 128:V], ptB[:, 0:nv])
            nc.vector.tensor_copy(utt1[0:nv, 0:128], ptC[0:nv, 0:128])
            nc.vector.tensor_copy(utt1[0:nv, 128:V], ptD[0:nv, 0:nv])
            nc.gpsimd.tensor_tensor(utt0[:, :], utt0[:, :], mask0, op=ALU.mult)
            nc.vector.tensor_tensor(at[0][:, :], utm[0][:, :], utt0[:, :], op=ALU.add)
            nc.gpsimd.tensor_tensor(utt1[:nv, :], utt1[:nv, :], mask1[:nv, :],
                                    op=ALU.mult)
            nc.vector.tensor_tensor(at[1][:nv, :], utm[1][:nv, :],
                                    utt1[:nv, :], op=ALU.add)

            for rt, (r0, nr) in enumerate([(0, 128), (128, V - 128)]):
                lt = at[rt]
                lc = smax.tile([128, V], F32, tag="lc")
                nc.vector.tensor_copy(lc[:nr, :], lt[:nr, :])
                mx = smax.tile([128, 8 * NR], F32, tag="mx")
                for r in range(NR):
                    nc.vector.max(mx[:nr, 8 * r: 8 * (r + 1)], lc[:nr, :])
                    if r < NR - 1:
                        nc.vector.match_replace(lc[:nr, :],
                                                mx[:nr, 8 * r: 8 * (r + 1)],
                                                lc[:nr, :], -1e30)
                nmx = smax.tile([128, 1], F32, tag="nmx")
                nc.vector.tensor_scalar_mul(nmx[:nr, :], mx[:nr, 0:1], -1.0)
                et = smax.tile([128, V], F32, tag="et")
                rsum = smax.tile([128, 1], F32, tag="rsum")
                nc.scalar.activation(et[:nr, :], lt[:nr, :], AF.Exp,
                                     bias=nmx[:nr, 0:1], scale=1.0,
                                     accum_out=rsum[:nr, 0:1])
                rec = smax.tile([128, 1], F32, tag="rec")
                nc.vector.reciprocal(rec[:nr, :], rsum[:nr, :])
                pt = smax.tile([128, V], F32, tag="pt")
                nc.vector.tensor_scalar_mul(pt[:nr, :], et[:nr, :], rec[:nr, 0:1])
                ot_ = smax.tile([128, V], F32, tag="ot")
                nc.vector.scalar_tensor_tensor(ot_[:nr, :], lt[:nr, :],
                                               mx[:nr, K - 1:K], pt[:nr, :],
                                               op0=ALU.is_ge, op1=ALU.mult)
                nc.sync.dma_start(out_d[r0:r0 + nr, :], ot_[:nr, :])
          if trace_scopes:
            nc.leave_named_scope(sc)

    nc.finalize()
    return nc


def _prep_inputs(inputs):
    """Host-side packing of the full inputs into one [128, NCOLS] per-core
    array: x^T (core's batch element) | conv weights | BN params | masks."""
    x = np.ascontiguousarray(inputs["x"], dtype=np.float32)

    def ctile(w):  # [cout, cin] -> [128, cin/128 * cout] packed per cin-tile
        wT = np.ascontiguousarray(np.asarray(w, np.float32).T)  # [cin, cout]
        cin, cout = wT.shape
        return wT.reshape(cin // 128, 128, cout).transpose(1, 0, 2).reshape(128, -1)

    def pcols(v):  # [cout] -> [128, cout/128]
        return np.asarray(v, np.float32).reshape(-1, 128).T

    parts = [ctile(inputs[f"w{k}"]) for k in range(5)]
    for k in range(4):
        parts.append(np.concatenate(
            [pcols(inputs[f"g{k}"]), pcols(inputs[f"be{k}"])], axis=1))
    parts.append(np.eye(128, dtype=np.float32))
    p_idx = np.arange(128, dtype=np.float32)[:, None]
    c_idx = np.arange(V, dtype=np.float32)[None, :]
    parts.append((c_idx < p_idx).astype(np.float32))          # strict-lower rows 0..127
    parts.append((c_idx < p_idx + 128).astype(np.float32))    # strict-lower rows 128..200
    parts.append((c_idx >= p_idx).astype(np.float32))         # upper-incl-diag rows 0..127
    parts.append((c_idx >= p_idx + 128).astype(np.float32))   # upper-incl-diag rows 128..200
    shared = np.concatenate(parts, axis=1)
    assert shared.shape == (128, NCOLS - 2 * V), shared.shape

    in_maps = []
    for c in range(N_CORES):
        xt = x[c].T.reshape(2, 128, V).transpose(1, 0, 2).reshape(128, 2 * V)
        in_maps.append({"pk": np.ascontiguousarray(
            np.concatenate([xt, shared], axis=1), np.float32)})
    return in_maps


class _Runner:
    """Cached PJRT executor for the bass module.

    Functionally the same axon path as bass_utils.run_bass_kernel_spmd
    (shard_map over 8 neuron devices + bass_exec custom call), but the
    jitted executable is built ONCE and reused — run_bass_kernel_spmd
    rebuilds the jax.jit closure per call, paying a full retrace/relower
    (~700ms) on every invocation. The donated-zero-output trick is also
    dropped: this kernel writes every element of its output, so the
    dummy output-shaped operands can be persistent device arrays instead
    of per-call zero uploads.
    """

    def __init__(self, nc):
        import jax
        from jax.sharding import Mesh, PartitionSpec, NamedSharding
        from jax.experimental.shard_map import shard_map
        from concourse.bass2jax import (
            _bass_exec_p,
            partition_id_tensor,
            install_neuronx_cc_hook,
        )

        install_neuronx_cc_hook()
        self.jax = jax
        self.nc = nc
        if nc.dbg_addr is not None and nc.dbg_callbacks:
            raise RuntimeError("dbg callbacks unsupported under axon")
        self.dbg_name = nc.dbg_addr.name if nc.dbg_addr is not None else None

        partition_name = (
            nc.partition_id_tensor.name if nc.partition_id_tensor else None
        )
        in_names, out_names, out_avals, zero_shapes = [], [], [], []
        for alloc in nc.m.functions[0].allocations:
            if not isinstance(alloc, mybir.MemoryLocationSet):
                continue
            name = alloc.memorylocations[0].name
            if alloc.kind == "ExternalInput":
                if name != partition_name:
                    in_names.append(name)
            elif alloc.kind == "ExternalOutput":
                out_names.append(name)
                shape = tuple(alloc.tensor_shape)
                dtype = mybir.dt.np(alloc.dtype)
                out_avals.append(jax.core.ShapedArray(shape, dtype))
                zero_shapes.append((shape, dtype))
        n_params = len(in_names)
        n_outs = len(out_avals)
        all_in_names = list(in_names) + list(out_names)
        if partition_name is not None:
            all_in_names.append(partition_name)
        self.in_names, self.out_names, self.out_avals = in_names, out_names, out_avals

        devices = jax.devices()[:N_CORES]
        assert len(devices) == N_CORES
        self.mesh = Mesh(np.asarray(devices), ("core",))
        self.sharding = NamedSharding(self.mesh, PartitionSpec("core"))

        def _body(*args):
            operands = list(args)
            if partition_name is not None:
                operands.append(partition_id_tensor())
            return tuple(
                _bass_exec_p.bind(
                    *operands,
                    out_avals=tuple(out_avals),
                    in_names=tuple(all_in_names),
                    out_names=tuple(out_names),
                    lowering_input_output_aliases=(),
                    sim_require_finite=True,
                    sim_require_nnan=True,
                    nc=nc,
                )
            )

        self._sharded = jax.jit(
            shard_map(
                _body,
                mesh=self.mesh,
                in_specs=(PartitionSpec("core"),) * (n_params + n_outs),
                out_specs=(PartitionSpec("core"),) * n_outs,
                check_rep=False,
            ),
            keep_unused=True,
        )
        # persistent dummy operands for the output slots (never read: the
        # kernel fully writes its outputs)
        self._zeros = [
            jax.device_put(
                np.zeros((N_CORES * s[0],) + tuple(s[1:]), dt), self.sharding
            )
            for (s, dt) in zero_shapes
        ]

    def concat_inputs(self, in_maps):
        if self.dbg_name is not None:
            dbg = np.zeros((1, 2), np.uint32)
            in_maps = [{**m, self.dbg_name: dbg} for m in in_maps]
        return [
            np.concatenate([np.asarray(m[nm]) for m in in_maps], axis=0)
            for nm in self.in_names
        ]

    def put_inputs(self, concat_in):
        return [self.jax.device_put(a, self.sharding) for a in concat_in]

    def dispatch(self, dev_in):
        """one kernel execution on the 8 cores (async; returns device arrays)"""
        return self._sharded(*dev_in, *self._zeros)

    def run_full(self, in_maps):
        outs = self.dispatch(self.put_inputs(self.concat_inputs(in_maps)))
        n = N_CORES
        return [
            {
                nm: np.asarray(outs[i]).reshape(n, *self.out_avals[i].shape)[c]
                for i, nm in enumerate(self.out_names)
            }
            for c in range(n)
        ]


_RUNNER = None


def _get_runner():
    global _RUNNER
    if _RUNNER is None:
        _RUNNER = _Runner(_build_nc())
    return _RUNNER


def kernel(**inputs):
    r = _get_runner()
    res = r.run_full(_prep_inputs(inputs))
    return np.stack([res[c]["outb"] for c in range(N_CORES)], axis=0)
